# revision 26
# baseline (speedup 1.0000x reference)
"""GCMC layer (gnn_message_passing) Bass kernel for 8 Trainium2 NeuronCores.

Strategy (dest-sharded, no collectives):
  out_dis[m, r, :] = ci_dis[m] * (S_dis[r][m] @ Wfc_r) + fc_b
  where S_dis[r][m] = sum_{edges e of rating r with dst=m} x_drug[src[e]]
        x_drug[n]   = cj_drug[n] * drug_feat[n]      (bf16 gather table)
        Wfc_r       = (sum_b att[r,b]*basis[b]) @ fc_w    [F, OUT]
  (and symmetrically for the reverse direction dis->drug)

  - Host sorts edges of each (direction, rating) by destination, shards
    destinations across 8 cores, and lays out per-dest-tile (128 dests)
    edge chunks of 128, padded to a static chunk count C with edges that
    point at an all-zero table row.
  - Launch 1: each core scales its 1/8 slice of node features by cj -> bf16.
    Host concatenates the slices into full gather tables.
  - Launch 2 (main): per dest tile: indirect-DMA gather of message rows
    [128, C, F], one-hot P = is_equal(dstloc, iota) on DVE, TensorE
    accumulates ZT[f, d] += M_chunk.T @ P_chunk in PSUM (the segment sum),
    second matmul ZT.T @ Wfc_r into a per-tile [128, R*OUT] PSUM bank,
    then one scalar_tensor_tensor applies ci scale + bias, and the result
    is stored contiguously in the final [node, r, out] layout.
"""

import json

import numpy as np
import ml_dtypes

import concourse.bass as bass
import concourse.mybir as mybir
import concourse.tile as tile
from concourse.bass_utils import run_bass_kernel_spmd

BF16 = ml_dtypes.bfloat16


# ----------------------------------------------------------------------
# Workaround: the staged walrus rejects >1 sync wait per instruction
# ("Too many sync wait commands") while the Tile scheduler emits multi-wait
# instructions.  Split extra waits into standalone EventSemaphore
# instructions right before the owning instruction (same engine queue, so
# semantics are identical: all waits are pre-conditions).
# ----------------------------------------------------------------------

def _split_multiwaits(bir: bytes) -> bytes:
    j = json.loads(bir)
    for fn in j["functions"]:
        for blk in fn["blocks"]:
            out = []
            k = 0
            for ins in blk["instructions"]:
                si = ins.get("sync_info") or {}
                waits = si.get("on_wait") or []
                if len(waits) > 1:
                    for w in waits[:-1]:
                        out.append({
                            "debug": ins.get("debug"),
                            "engine": ins["engine"],
                            "ins": [], "outs": [],
                            "name": f"{ins['name']}-ws{k}",
                            "opcode": "EventSemaphore",
                            "sync_info": {"on_update": [], "on_wait": [w]},
                        })
                        k += 1
                    si["on_wait"] = [waits[-1]]
                out.append(ins)
            blk["instructions"] = out
    return json.dumps(j).encode()


_orig_to_json_bytes = bass.Bass.to_json_bytes


def _patched_to_json_bytes(self):
    return _split_multiwaits(_orig_to_json_bytes(self))


bass.Bass.to_json_bytes = _patched_to_json_bytes

# ----- problem constants (hardcoded per contract) -----
N = 50000          # nodes per side
F = 128            # feature dim
R = 5              # ratings
E = 400000         # edges per rating per direction
OUT = 64           # output dim
NB = 2             # basis count
NCORES = 8

f32 = mybir.dt.float32
bf16 = mybir.dt.bfloat16
i32 = mybir.dt.int32


def _derived():
    npc = N // NCORES
    nt = (npc + 127) // 128
    npad = nt * 128
    tbl = ((N + 1 + 127) // 128) * 128  # >= N+1 so row N exists and is zero
    return npc, nt, npad, tbl


# ======================================================================
# Host-side edge preprocessing
# ======================================================================

def _prep_direction(src_all: np.ndarray, dst_all: np.ndarray, C: int | None):
    """For one direction: returns (srcidx, dstloc, maxc) where
    srcidx: int32 [NCORES, 128, R*NT*C]   gather indices into the src table
    dstloc: bf16  [NCORES, 128, R*NT*C]   dest offset (0..127) within tile
    Column (r*NT + t)*C + j, partition i  <->  edge slot j*128+i of tile t.
    If C is None, only computes the needed max chunk count.
    """
    NPC, NT, _, _ = _derived()
    ZROW = N
    per_core = []
    maxc = 1
    for r in range(R):
        order = np.argsort(dst_all[r], kind="stable")
        dst_s = dst_all[r][order].astype(np.int64)
        src_s = src_all[r][order].astype(np.int64)
        bounds = np.searchsorted(dst_s, np.arange(NCORES + 1) * NPC)
        for c in range(NCORES):
            lo, hi = bounds[c], bounds[c + 1]
            d = dst_s[lo:hi] - c * NPC
            s = src_s[lo:hi]
            tile_id = d >> 7
            tcnt = np.bincount(tile_id, minlength=NT)
            maxc = max(maxc, int((tcnt.max() + 127) // 128))
            if C is not None:
                per_core.append((r, c, d, s, tile_id, tcnt))
    if C is None:
        return None, None, maxc
    assert maxc <= C, f"need C>={maxc}, got {C}"

    srcidx = np.full((NCORES, R, NT, C * 128), ZROW, np.int32)
    dstloc = np.zeros((NCORES, R, NT, C * 128), np.float32)
    for (r, c, d, s, tile_id, tcnt) in per_core:
        tstart = np.zeros(NT + 1, np.int64)
        np.cumsum(tcnt, out=tstart[1:])
        rank = np.arange(len(d)) - tstart[tile_id]
        srcidx[c, r, tile_id, rank] = s
        dstloc[c, r, tile_id, rank] = (d & 127).astype(np.float32)
    # [c, r, NT, C, 128] -> [c, 128, r, NT, C] -> [c, 128, R*NT*C]
    srcidx = np.ascontiguousarray(
        srcidx.reshape(NCORES, R, NT, C, 128).transpose(0, 4, 1, 2, 3)
    ).reshape(NCORES, 128, R * NT * C)
    dstloc = np.ascontiguousarray(
        dstloc.reshape(NCORES, R, NT, C, 128).transpose(0, 4, 1, 2, 3)
    ).reshape(NCORES, 128, R * NT * C).astype(BF16)
    return srcidx, dstloc, maxc


# ======================================================================
# Launch 1: build bf16 gather tables (x = cj * feat), row-sharded
# ======================================================================

def build_prep_nc():
    NPC, NT, NPAD, TBL = _derived()
    nc = bass.Bass()
    feat_in = nc.dram_tensor("feat_slice", (2, NPAD, F), f32, kind="ExternalInput")
    # cj host-transposed to [2, 128, NT]: element [s, i, t] = cj[s, t*128+i]
    cj_in = nc.dram_tensor("cj_slice", (2, 128, NT), f32, kind="ExternalInput")
    x_out = nc.dram_tensor("x_slice", (2, NPAD, F), bf16, kind="ExternalOutput")

    with tile.TileContext(nc) as tc:
        with (
            tc.tile_pool(name="cj", bufs=1) as cjp,
            tc.tile_pool(name="sb", bufs=6) as sb,
        ):
            cj_sb = cjp.tile([128, 2 * NT], f32, tag="cj")
            nc.sync.dma_start(
                out=cj_sb[:].rearrange("p (s t) -> p s t", s=2),
                in_=cj_in[:, :, :].rearrange("s p t -> p s t"))
            # absorber: advance DVE's clock past the cj DMA so later consumers
            # need only one wait (walrus allows a single sync wait per compute
            # instruction and the scheduler doesn't always split).
            scratch = cjp.tile([128, 1], f32, tag="scratch")
            nc.vector.tensor_copy(out=scratch[:], in_=cj_sb[:, :1])
            for side in range(2):
                for t in range(NT):
                    rows = slice(t * 128, (t + 1) * 128)
                    ft = sb.tile([128, F], f32, tag="ft")
                    nc.sync.dma_start(out=ft[:], in_=feat_in[side, rows, :])
                    xt = sb.tile([128, F], bf16, tag="xt")
                    c0 = side * NT + t
                    # tensor_tensor (not tensor_scalar): the TS ISA struct only
                    # fits one sync wait and the scheduler may attach two.
                    nc.vector.tensor_tensor(
                        out=xt[:], in0=ft[:],
                        in1=cj_sb[:, c0:c0 + 1].to_broadcast([128, F]),
                        op=mybir.AluOpType.mult,
                    )
                    nc.sync.dma_start(out=x_out[side, rows, :], in_=xt[:])
    return nc


# ======================================================================
# Launch 2: main kernel
# ======================================================================

DEBUG_TAPS = False


def build_main_nc(C: int):
    NPC, NT, NPAD, TBL = _derived()
    nc = bass.Bass()
    dbg = {}
    if DEBUG_TAPS:
        dbg["m"] = nc.dram_tensor("dbg_m", (128, C * 128), bf16, kind="ExternalOutput")
        dbg["p"] = nc.dram_tensor("dbg_p", (128, C * 128), bf16, kind="ExternalOutput")
        dbg["zt"] = nc.dram_tensor("dbg_zt", (128, 128), bf16, kind="ExternalOutput")
        dbg["wfc"] = nc.dram_tensor("dbg_wfc", (128, R * OUT), bf16, kind="ExternalOutput")
        dbg["o2"] = nc.dram_tensor("dbg_o2", (128, R * OUT), f32, kind="ExternalOutput")
    x_drug = nc.dram_tensor("x_drug", (TBL, F), bf16, kind="ExternalInput")
    x_dis = nc.dram_tensor("x_dis", (TBL, F), bf16, kind="ExternalInput")
    srcidx_in = nc.dram_tensor("srcidx", (2, 128, R * NT * C), i32, kind="ExternalInput")
    dstloc_in = nc.dram_tensor("dstloc", (2, 128, R * NT * C), bf16, kind="ExternalInput")
    # ci host-transposed to [2, 128, NT]
    ci_in = nc.dram_tensor("ci_pad", (2, 128, NT), f32, kind="ExternalInput")
    att_in = nc.dram_tensor("att", (R, NB), f32, kind="ExternalInput")
    basis_in = nc.dram_tensor("basis", (NB, F, F), f32, kind="ExternalInput")
    fcw_in = nc.dram_tensor("fc_w", (F, OUT), f32, kind="ExternalInput")
    fcb_in = nc.dram_tensor("fc_b", (OUT,), f32, kind="ExternalInput")
    out = nc.dram_tensor("out_part", (2, NPAD, R, OUT), f32, kind="ExternalOutput")

    iota_np = np.broadcast_to(np.arange(128, dtype=np.float32), (128, 128))
    iota_c = nc.inline_tensor(np.ascontiguousarray(iota_np), "iota_c")
    ident_c = nc.inline_tensor(np.eye(128, dtype=np.float32), "ident_c")
    ones_c = nc.inline_tensor(np.ones((1, 128), dtype=np.float32), "ones_c")

    eq = mybir.AluOpType.is_equal
    mult = mybir.AluOpType.mult
    add = mybir.AluOpType.add

    with tile.TileContext(nc) as tc:
        with (
            tc.tile_pool(name="const", bufs=1) as cp,
            tc.tile_pool(name="sb", bufs=4) as sb,
            tc.tile_pool(name="idxp", bufs=2) as idxp,
            tc.tile_pool(name="ps", bufs=4, space="PSUM") as ps,
            tc.tile_pool(name="ps2", bufs=2, space="PSUM") as ps2,
        ):
            # ---------- consts ----------
            iota_f = cp.tile([128, 128], f32, tag="iotaf")
            nc.sync.dma_start(out=iota_f[:], in_=iota_c[:, :])
            iota_t = cp.tile([128, 128], bf16, tag="iota")
            nc.vector.tensor_copy(out=iota_t[:], in_=iota_f[:])
            ident_t = cp.tile([128, 128], f32, tag="ident")
            nc.sync.dma_start(out=ident_t[:], in_=ident_c[:, :])
            ones_f32 = cp.tile([1, 128], f32, tag="ones32")
            nc.sync.dma_start(out=ones_f32[:], in_=ones_c[:, :])
            fcw_t = cp.tile([128, OUT], f32, tag="fcw")
            nc.sync.dma_start(out=fcw_t[:], in_=fcw_in[:, :])
            fcb_row = cp.tile([1, OUT], f32, tag="fcbrow")
            nc.sync.dma_start(out=fcb_row[:], in_=fcb_in[None, :])
            att_row = cp.tile([1, R * NB], f32, tag="attrow")
            nc.sync.dma_start(out=att_row[:],
                              in_=att_in[:, :].rearrange("r b -> () (r b)"))

            # ---------- W prep ----------
            # broadcast att and bias across partitions via K=1 matmuls
            attb_ps = ps2.tile([128, R * NB], f32, tag="o2")
            nc.tensor.matmul(out=attb_ps[:], lhsT=ones_f32[:], rhs=att_row[:],
                             start=True, stop=True)
            att_b = cp.tile([128, R * NB], f32, tag="attb")
            nc.vector.tensor_copy(out=att_b[:], in_=attb_ps[:])

            biasb_ps = ps2.tile([128, OUT], f32, tag="o2")
            nc.tensor.matmul(out=biasb_ps[:], lhsT=ones_f32[:], rhs=fcb_row[:],
                             start=True, stop=True)
            bias5 = cp.tile([128, R * OUT], f32, tag="bias5")
            for r in range(R):
                nc.vector.tensor_copy(out=bias5[:, r * OUT:(r + 1) * OUT],
                                      in_=biasb_ps[:])

            # basis[b] transposed: [e, f]
            bT = []
            for b in range(NB):
                bt_in = sb.tile([128, 128], f32, tag="bload")
                nc.sync.dma_start(out=bt_in[:], in_=basis_in[b, :, :])
                bt_ps = ps.tile([128, 128], f32, tag="zt")
                nc.tensor.transpose(out=bt_ps[:], in_=bt_in[:], identity=ident_t[:])
                bt_sb = cp.tile([128, 128], f32, tag=f"bT{b}")
                nc.vector.tensor_copy(out=bt_sb[:], in_=bt_ps[:])
                bT.append(bt_sb)

            wfc = cp.tile([128, R * OUT], bf16, tag="wfc")
            for r in range(R):
                wrt = sb.tile([128, 128], f32, tag="wrt")
                tmp = sb.tile([128, 128], f32, tag="wtmp")
                nc.vector.tensor_tensor(
                    out=tmp[:], in0=bT[1][:],
                    in1=att_b[:, 2 * r + 1:2 * r + 2].to_broadcast([128, 128]),
                    op=mult,
                )
                nc.vector.tensor_tensor(
                    out=wrt[:], in0=bT[0][:],
                    in1=att_b[:, 2 * r:2 * r + 1].to_broadcast([128, 128]),
                    op=mult,
                )
                nc.vector.tensor_tensor(out=wrt[:], in0=wrt[:], in1=tmp[:], op=add)
                wfc_ps = ps2.tile([128, OUT], f32, tag="o2")
                nc.tensor.matmul(out=wfc_ps[:], lhsT=wrt[:], rhs=fcw_t[:],
                                 start=True, stop=True)
                nc.scalar.copy(out=wfc[:, r * OUT:(r + 1) * OUT], in_=wfc_ps[:])

            # ---------- main loops ----------
            if DEBUG_TAPS:
                nc.sync.dma_start(out=dbg["wfc"][:, :], in_=wfc[:])
                dbg_o2_sb = cp.tile([128, R * OUT], f32, tag="dbgo2")
            ci_sb = cp.tile([128, 2 * NT], f32, tag="ci")
            nc.sync.dma_start(
                out=ci_sb[:].rearrange("p (s t) -> p s t", s=2),
                in_=ci_in[:, :, :].rearrange("s p t -> p s t"))
            for d in range(2):
                x_src = x_dis if d else x_drug
                idx_t = idxp.tile([128, R * NT * C], i32, tag="idx")
                nc.sync.dma_start(out=idx_t[:], in_=srcidx_in[d, :, :])
                dl_t = idxp.tile([128, R * NT * C], bf16, tag="dl")
                nc.sync.dma_start(out=dl_t[:], in_=dstloc_in[d, :, :])
                for t in range(NT):
                    rows = slice(t * 128, (t + 1) * 128)
                    ci_col = d * NT + t
                    o2 = ps2.tile([128, R * OUT], f32, tag="o2")
                    for r in range(R):
                        col0 = (r * NT + t) * C
                        # the HW indirect DMA supports exactly one gathered row
                        # per partition per instruction -> one gather per chunk
                        p_t = sb.tile([128, C * 128], bf16, tag="p")
                        nc.vector.tensor_tensor(
                            out=p_t[:].rearrange("p (c f) -> p c f", c=C),
                            in0=dl_t[:, col0:col0 + C][:, :, None].to_broadcast(
                                [128, C, 128]),
                            in1=iota_t[:, None, :].to_broadcast([128, C, 128]),
                            op=eq,
                        )
                        zt = ps.tile([128, 128], f32, tag="zt")
                        for j in range(C):
                            m_t = sb.tile([128, 128], bf16, tag="m")
                            nc.gpsimd.indirect_dma_start(
                                out=m_t[:], out_offset=None,
                                in_=x_src[:, :],
                                in_offset=bass.IndirectOffsetOnAxis(
                                    ap=idx_t[:, col0 + j:col0 + j + 1], axis=0),
                            )
                            nc.tensor.matmul(
                                out=zt[:], lhsT=m_t[:],
                                rhs=p_t[:, j * 128:(j + 1) * 128],
                                start=(j == 0), stop=(j == C - 1),
                            )
                        zt_sb = sb.tile([128, 128], bf16, tag="ztsb")
                        nc.scalar.copy(out=zt_sb[:], in_=zt[:])
                        nc.tensor.matmul(
                            out=o2[:, r * OUT:(r + 1) * OUT], lhsT=zt_sb[:],
                            rhs=wfc[:, r * OUT:(r + 1) * OUT],
                            start=True, stop=True,
                        )
                        if DEBUG_TAPS and d == 0 and t == 0 and r == 0:
                            nc.sync.dma_start(out=dbg["p"][:, :], in_=p_t[:])
                            nc.sync.dma_start(out=dbg["zt"][:, :], in_=zt_sb[:])
                    if DEBUG_TAPS and d == 0 and t == 0:
                        nc.vector.tensor_copy(out=dbg_o2_sb[:], in_=o2[:])
                        nc.sync.dma_start(out=dbg["o2"][:, :], in_=dbg_o2_sb[:])
                    ob = sb.tile([128, R * OUT], f32, tag="ob")
                    nc.vector.tensor_tensor(
                        out=ob[:], in0=o2[:],
                        in1=ci_sb[:, ci_col:ci_col + 1].to_broadcast(
                            [128, R * OUT]),
                        op=mult,
                    )
                    nc.vector.tensor_tensor(
                        out=ob[:], in0=ob[:], in1=bias5[:], op=add)
                    nc.sync.dma_start(
                        out=out[d, rows, :, :].rearrange("p r o -> p (r o)"),
                        in_=ob[:],
                    )
    return nc


# ======================================================================
# kernel entry
# ======================================================================

_cache: dict = {}


def kernel(drug_feat, dis_feat, cj_drug, ci_drug, cj_dis, ci_dis,
           att, basis, fc_w, fc_b, edge_drug, edge_dis):
    NPC, NT, NPAD, TBL = _derived()
    drug_feat = np.asarray(drug_feat, np.float32)
    dis_feat = np.asarray(dis_feat, np.float32)
    cj_drug = np.asarray(cj_drug, np.float32)
    ci_drug = np.asarray(ci_drug, np.float32)
    cj_dis = np.asarray(cj_dis, np.float32)
    ci_dis = np.asarray(ci_dis, np.float32)
    att = np.asarray(att, np.float32)
    basis = np.asarray(basis, np.float32)
    fc_w = np.asarray(fc_w, np.float32)
    fc_b = np.asarray(fc_b, np.float32)
    edge_drug = np.asarray(edge_drug, np.int32)
    edge_dis = np.asarray(edge_dis, np.int32)

    # ---- host preprocessing: edge sort/shard (index manipulation only) ----
    # direction 0: drug -> dis (dest = dis), direction 1: dis -> drug
    _, _, c0 = _prep_direction(edge_drug, edge_dis, None)
    _, _, c1 = _prep_direction(edge_dis, edge_drug, None)
    C = max(c0, c1)
    src0, dl0, _ = _prep_direction(edge_drug, edge_dis, C)
    src1, dl1, _ = _prep_direction(edge_dis, edge_drug, C)

    # ---- launch 1: build gather tables ----
    if "prep" not in _cache:
        _cache["prep"] = build_prep_nc()
    nc1 = _cache["prep"]

    in_maps1 = []
    for c in range(NCORES):
        rows = slice(c * NPC, (c + 1) * NPC)
        feat_slice = np.zeros((2, NPAD, F), np.float32)
        feat_slice[0, :NPC] = drug_feat[rows]
        feat_slice[1, :NPC] = dis_feat[rows]
        cj_slice = np.zeros((2, NPAD), np.float32)
        cj_slice[0, :NPC] = cj_drug[rows]
        cj_slice[1, :NPC] = cj_dis[rows]
        cj_slice = np.ascontiguousarray(
            cj_slice.reshape(2, NT, 128).transpose(0, 2, 1))
        in_maps1.append({"feat_slice": feat_slice, "cj_slice": cj_slice})
    res1 = run_bass_kernel_spmd(nc1, in_maps1, core_ids=list(range(NCORES)))
    xs = [r["x_slice"] for r in res1.results]
    x_drug_full = np.zeros((TBL, F), BF16)
    x_dis_full = np.zeros((TBL, F), BF16)
    for c in range(NCORES):
        rows = slice(c * NPC, (c + 1) * NPC)
        x_drug_full[rows] = xs[c][0, :NPC]
        x_dis_full[rows] = xs[c][1, :NPC]

    # ---- launch 2: main ----
    key = ("main", C)
    if key not in _cache:
        _cache[key] = build_main_nc(C)
    nc2 = _cache[key]

    in_maps2 = []
    for c in range(NCORES):
        rows = slice(c * NPC, (c + 1) * NPC)
        srcidx = np.stack([src0[c], src1[c]], axis=0)
        dstloc = np.stack([dl0[c], dl1[c]], axis=0)
        ci_pad = np.zeros((2, NPAD), np.float32)
        ci_pad[0, :NPC] = ci_dis[rows]    # dir 0 dest = dis
        ci_pad[1, :NPC] = ci_drug[rows]   # dir 1 dest = drug
        ci_pad = np.ascontiguousarray(
            ci_pad.reshape(2, NT, 128).transpose(0, 2, 1))
        in_maps2.append({
            "x_drug": x_drug_full, "x_dis": x_dis_full,
            "srcidx": srcidx, "dstloc": dstloc, "ci_pad": ci_pad,
            "att": att, "basis": basis, "fc_w": fc_w, "fc_b": fc_b,
        })
    res2 = run_bass_kernel_spmd(nc2, in_maps2, core_ids=list(range(NCORES)))

    out_dis = np.concatenate(
        [r["out_part"][0, :NPC] for r in res2.results], axis=0)
    out_drug = np.concatenate(
        [r["out_part"][1, :NPC] for r in res2.results], axis=0)
    return out_drug.astype(np.float32), out_dis.astype(np.float32)


# revision 28
# speedup vs baseline: 365.2765x; 365.2765x over previous
"""GCMC layer (gnn_message_passing) Bass kernel for 8 Trainium2 NeuronCores.

Strategy (dest-sharded, no collectives):
  out_dis[m, r, :] = ci_dis[m] * (S_dis[r][m] @ Wfc_r) + fc_b
  where S_dis[r][m] = sum_{edges e of rating r with dst=m} x_drug[src[e]]
        x_drug[n]   = cj_drug[n] * drug_feat[n]      (bf16 gather table)
        Wfc_r       = (sum_b att[r,b]*basis[b]) @ fc_w    [F, OUT]
  (and symmetrically for the reverse direction dis->drug)

  - Host sorts edges of each (direction, rating) by destination, shards
    destinations across 8 cores, and lays out per-dest-tile (128 dests)
    edge chunks of 128, padded to a static chunk count C with edges that
    point at an all-zero table row.
  - Launch 1: each core scales its 1/8 slice of node features by cj -> bf16.
    Host concatenates the slices into full gather tables.
  - Launch 2 (main): per dest tile: indirect-DMA gather of message rows
    [128, C, F], one-hot P = is_equal(dstloc, iota) on DVE, TensorE
    accumulates ZT[f, d] += M_chunk.T @ P_chunk in PSUM (the segment sum),
    second matmul ZT.T @ Wfc_r into a per-tile [128, R*OUT] PSUM bank,
    then one scalar_tensor_tensor applies ci scale + bias, and the result
    is stored contiguously in the final [node, r, out] layout.
"""

import json
import os
import time

import numpy as np
import ml_dtypes

_VERBOSE = os.environ.get("KERNEL_VERBOSE", "0") == "1"


def _tlog(msg, t0=[None]):
    if _VERBOSE:
        now = time.time()
        dt = 0.0 if t0[0] is None else now - t0[0]
        t0[0] = now
        print(f"[kernel +{dt:6.2f}s] {msg}", flush=True)

import concourse.bass as bass
import concourse.mybir as mybir
import concourse.tile as tile
from concourse.bass_utils import run_bass_kernel_spmd

BF16 = ml_dtypes.bfloat16


# ----------------------------------------------------------------------
# Workaround: the staged walrus rejects >1 sync wait per instruction
# ("Too many sync wait commands") while the Tile scheduler emits multi-wait
# instructions.  Split extra waits into standalone EventSemaphore
# instructions right before the owning instruction (same engine queue, so
# semantics are identical: all waits are pre-conditions).
# ----------------------------------------------------------------------

def _split_multiwaits(bir: bytes) -> bytes:
    j = json.loads(bir)
    for fn in j["functions"]:
        for blk in fn["blocks"]:
            out = []
            k = 0
            for ins in blk["instructions"]:
                si = ins.get("sync_info") or {}
                waits = si.get("on_wait") or []
                if len(waits) > 1:
                    for w in waits[:-1]:
                        out.append({
                            "debug": ins.get("debug"),
                            "engine": ins["engine"],
                            "ins": [], "outs": [],
                            "name": f"{ins['name']}-ws{k}",
                            "opcode": "EventSemaphore",
                            "sync_info": {"on_update": [], "on_wait": [w]},
                        })
                        k += 1
                    si["on_wait"] = [waits[-1]]
                out.append(ins)
            blk["instructions"] = out
    return json.dumps(j).encode()


_orig_to_json_bytes = bass.Bass.to_json_bytes


def _patched_to_json_bytes(self):
    return _split_multiwaits(_orig_to_json_bytes(self))


bass.Bass.to_json_bytes = _patched_to_json_bytes

# ----- problem constants (hardcoded per contract) -----
N = 50000          # nodes per side
F = 128            # feature dim
R = 5              # ratings
E = 400000         # edges per rating per direction
OUT = 64           # output dim
NB = 2             # basis count
NCORES = 8

f32 = mybir.dt.float32
bf16 = mybir.dt.bfloat16
i32 = mybir.dt.int32


def _derived():
    npc = N // NCORES
    nt = (npc + 127) // 128
    npad = nt * 128
    tbl = ((N + 1 + 127) // 128) * 128  # >= N+1 so row N exists and is zero
    return npc, nt, npad, tbl


# ======================================================================
# Host-side edge preprocessing
# ======================================================================

def _prep_direction(src_all: np.ndarray, dst_all: np.ndarray, C: int | None):
    """For one direction: returns (srcidx, dstloc, maxc) where
    srcidx: int32 [NCORES, 128, R*NT*C]   gather indices into the src table
    dstloc: bf16  [NCORES, 128, R*NT*C]   dest offset (0..127) within tile
    Column (r*NT + t)*C + j, partition i  <->  edge slot j*128+i of tile t.
    If C is None, only computes the needed max chunk count.
    """
    NPC, NT, _, _ = _derived()
    ZROW = N
    per_core = []
    maxc = 1
    for r in range(R):
        order = np.argsort(dst_all[r], kind="stable")
        dst_s = dst_all[r][order].astype(np.int64)
        src_s = src_all[r][order].astype(np.int64)
        bounds = np.searchsorted(dst_s, np.arange(NCORES + 1) * NPC)
        for c in range(NCORES):
            lo, hi = bounds[c], bounds[c + 1]
            d = dst_s[lo:hi] - c * NPC
            s = src_s[lo:hi]
            tile_id = d >> 7
            tcnt = np.bincount(tile_id, minlength=NT)
            maxc = max(maxc, int((tcnt.max() + 127) // 128))
            if C is not None:
                per_core.append((r, c, d, s, tile_id, tcnt))
    if C is None:
        return None, None, maxc
    assert maxc <= C, f"need C>={maxc}, got {C}"

    srcidx = np.full((NCORES, R, NT, C * 128), ZROW, np.int32)
    dstloc = np.zeros((NCORES, R, NT, C * 128), np.float32)
    for (r, c, d, s, tile_id, tcnt) in per_core:
        tstart = np.zeros(NT + 1, np.int64)
        np.cumsum(tcnt, out=tstart[1:])
        rank = np.arange(len(d)) - tstart[tile_id]
        srcidx[c, r, tile_id, rank] = s
        dstloc[c, r, tile_id, rank] = (d & 127).astype(np.float32)
    # [c, r, NT, C, 128] -> [c, 128, r, NT, C] -> [c, 128, R*NT*C]
    srcidx = np.ascontiguousarray(
        srcidx.reshape(NCORES, R, NT, C, 128).transpose(0, 4, 1, 2, 3)
    ).reshape(NCORES, 128, R * NT * C)
    dstloc = np.ascontiguousarray(
        dstloc.reshape(NCORES, R, NT, C, 128).transpose(0, 4, 1, 2, 3)
    ).reshape(NCORES, 128, R * NT * C).astype(BF16)
    return srcidx, dstloc, maxc


# ======================================================================
# Launch 1: build bf16 gather tables (x = cj * feat), row-sharded
# ======================================================================

def build_prep_nc():
    NPC, NT, NPAD, TBL = _derived()
    nc = bass.Bass()
    feat_in = nc.dram_tensor("feat_slice", (2, NPAD, F), f32, kind="ExternalInput")
    # cj host-transposed to [2, 128, NT]: element [s, i, t] = cj[s, t*128+i]
    cj_in = nc.dram_tensor("cj_slice", (2, 128, NT), f32, kind="ExternalInput")
    x_out = nc.dram_tensor("x_slice", (2, NPAD, F), bf16, kind="ExternalOutput")

    with tile.TileContext(nc) as tc:
        with (
            tc.tile_pool(name="cj", bufs=1) as cjp,
            tc.tile_pool(name="sb", bufs=6) as sb,
        ):
            cj_sb = cjp.tile([128, 2 * NT], f32, tag="cj")
            nc.sync.dma_start(
                out=cj_sb[:].rearrange("p (s t) -> p s t", s=2),
                in_=cj_in[:, :, :].rearrange("s p t -> p s t"))
            # absorber: advance DVE's clock past the cj DMA so later consumers
            # need only one wait (walrus allows a single sync wait per compute
            # instruction and the scheduler doesn't always split).
            scratch = cjp.tile([128, 1], f32, tag="scratch")
            nc.vector.tensor_copy(out=scratch[:], in_=cj_sb[:, :1])
            for side in range(2):
                for t in range(NT):
                    rows = slice(t * 128, (t + 1) * 128)
                    ft = sb.tile([128, F], f32, tag="ft")
                    nc.sync.dma_start(out=ft[:], in_=feat_in[side, rows, :])
                    xt = sb.tile([128, F], bf16, tag="xt")
                    c0 = side * NT + t
                    # tensor_tensor (not tensor_scalar): the TS ISA struct only
                    # fits one sync wait and the scheduler may attach two.
                    nc.vector.tensor_tensor(
                        out=xt[:], in0=ft[:],
                        in1=cj_sb[:, c0:c0 + 1].to_broadcast([128, F]),
                        op=mybir.AluOpType.mult,
                    )
                    nc.sync.dma_start(out=x_out[side, rows, :], in_=xt[:])
    return nc


# ======================================================================
# Launch 2: main kernel
# ======================================================================

DEBUG_TAPS = False


def build_main_nc(C: int):
    NPC, NT, NPAD, TBL = _derived()
    nc = bass.Bass()
    dbg = {}
    if DEBUG_TAPS:
        dbg["m"] = nc.dram_tensor("dbg_m", (128, C * 128), bf16, kind="ExternalOutput")
        dbg["p"] = nc.dram_tensor("dbg_p", (128, C * 128), bf16, kind="ExternalOutput")
        dbg["zt"] = nc.dram_tensor("dbg_zt", (128, 128), bf16, kind="ExternalOutput")
        dbg["wfc"] = nc.dram_tensor("dbg_wfc", (128, R * OUT), bf16, kind="ExternalOutput")
        dbg["o2"] = nc.dram_tensor("dbg_o2", (128, R * OUT), f32, kind="ExternalOutput")
    x_drug = nc.dram_tensor("x_drug", (TBL, F), bf16, kind="ExternalInput")
    x_dis = nc.dram_tensor("x_dis", (TBL, F), bf16, kind="ExternalInput")
    srcidx_in = nc.dram_tensor("srcidx", (2, 128, R * NT * C), i32, kind="ExternalInput")
    dstloc_in = nc.dram_tensor("dstloc", (2, 128, R * NT * C), bf16, kind="ExternalInput")
    # ci host-transposed to [2, 128, NT]
    ci_in = nc.dram_tensor("ci_pad", (2, 128, NT), f32, kind="ExternalInput")
    att_in = nc.dram_tensor("att", (R, NB), f32, kind="ExternalInput")
    basis_in = nc.dram_tensor("basis", (NB, F, F), f32, kind="ExternalInput")
    fcw_in = nc.dram_tensor("fc_w", (F, OUT), f32, kind="ExternalInput")
    fcb_in = nc.dram_tensor("fc_b", (OUT,), f32, kind="ExternalInput")
    out = nc.dram_tensor("out_part", (2, NPAD, R, OUT), f32, kind="ExternalOutput")

    iota_np = np.broadcast_to(np.arange(128, dtype=np.float32), (128, 128))
    iota_c = nc.inline_tensor(np.ascontiguousarray(iota_np), "iota_c")
    ident_c = nc.inline_tensor(np.eye(128, dtype=np.float32), "ident_c")
    ones_c = nc.inline_tensor(np.ones((1, 128), dtype=np.float32), "ones_c")

    eq = mybir.AluOpType.is_equal
    mult = mybir.AluOpType.mult
    add = mybir.AluOpType.add

    with tile.TileContext(nc) as tc:
        with (
            tc.tile_pool(name="const", bufs=1) as cp,
            tc.tile_pool(name="sb", bufs=4) as sb,
            tc.tile_pool(name="idxp", bufs=2) as idxp,
            tc.tile_pool(name="ps", bufs=4, space="PSUM") as ps,
            tc.tile_pool(name="ps2", bufs=2, space="PSUM") as ps2,
        ):
            # ---------- consts ----------
            iota_f = cp.tile([128, 128], f32, tag="iotaf")
            nc.sync.dma_start(out=iota_f[:], in_=iota_c[:, :])
            iota_t = cp.tile([128, 128], bf16, tag="iota")
            nc.vector.tensor_copy(out=iota_t[:], in_=iota_f[:])
            ident_t = cp.tile([128, 128], f32, tag="ident")
            nc.sync.dma_start(out=ident_t[:], in_=ident_c[:, :])
            ones_f32 = cp.tile([1, 128], f32, tag="ones32")
            nc.sync.dma_start(out=ones_f32[:], in_=ones_c[:, :])
            fcw_t = cp.tile([128, OUT], f32, tag="fcw")
            nc.sync.dma_start(out=fcw_t[:], in_=fcw_in[:, :])
            fcb_row = cp.tile([1, OUT], f32, tag="fcbrow")
            nc.sync.dma_start(out=fcb_row[:], in_=fcb_in[None, :])
            att_row = cp.tile([1, R * NB], f32, tag="attrow")
            nc.sync.dma_start(out=att_row[:],
                              in_=att_in[:, :].rearrange("r b -> () (r b)"))

            # ---------- W prep ----------
            # broadcast att and bias across partitions via K=1 matmuls
            attb_ps = ps2.tile([128, R * NB], f32, tag="o2")
            nc.tensor.matmul(out=attb_ps[:], lhsT=ones_f32[:], rhs=att_row[:],
                             start=True, stop=True)
            att_b = cp.tile([128, R * NB], f32, tag="attb")
            nc.vector.tensor_copy(out=att_b[:], in_=attb_ps[:])

            biasb_ps = ps2.tile([128, OUT], f32, tag="o2")
            nc.tensor.matmul(out=biasb_ps[:], lhsT=ones_f32[:], rhs=fcb_row[:],
                             start=True, stop=True)
            bias5 = cp.tile([128, R * OUT], f32, tag="bias5")
            for r in range(R):
                nc.vector.tensor_copy(out=bias5[:, r * OUT:(r + 1) * OUT],
                                      in_=biasb_ps[:])

            # basis[b] transposed: [e, f]
            bT = []
            for b in range(NB):
                bt_in = sb.tile([128, 128], f32, tag="bload")
                nc.sync.dma_start(out=bt_in[:], in_=basis_in[b, :, :])
                bt_ps = ps.tile([128, 128], f32, tag="zt")
                nc.tensor.transpose(out=bt_ps[:], in_=bt_in[:], identity=ident_t[:])
                bt_sb = cp.tile([128, 128], f32, tag=f"bT{b}")
                nc.vector.tensor_copy(out=bt_sb[:], in_=bt_ps[:])
                bT.append(bt_sb)

            wfc = cp.tile([128, R * OUT], bf16, tag="wfc")
            for r in range(R):
                wrt = sb.tile([128, 128], f32, tag="wrt")
                tmp = sb.tile([128, 128], f32, tag="wtmp")
                nc.vector.tensor_tensor(
                    out=tmp[:], in0=bT[1][:],
                    in1=att_b[:, 2 * r + 1:2 * r + 2].to_broadcast([128, 128]),
                    op=mult,
                )
                nc.vector.tensor_tensor(
                    out=wrt[:], in0=bT[0][:],
                    in1=att_b[:, 2 * r:2 * r + 1].to_broadcast([128, 128]),
                    op=mult,
                )
                nc.vector.tensor_tensor(out=wrt[:], in0=wrt[:], in1=tmp[:], op=add)
                wfc_ps = ps2.tile([128, OUT], f32, tag="o2")
                nc.tensor.matmul(out=wfc_ps[:], lhsT=wrt[:], rhs=fcw_t[:],
                                 start=True, stop=True)
                nc.scalar.copy(out=wfc[:, r * OUT:(r + 1) * OUT], in_=wfc_ps[:])

            # ---------- main loops ----------
            if DEBUG_TAPS:
                nc.sync.dma_start(out=dbg["wfc"][:, :], in_=wfc[:])
                dbg_o2_sb = cp.tile([128, R * OUT], f32, tag="dbgo2")
            ci_sb = cp.tile([128, 2 * NT], f32, tag="ci")
            nc.sync.dma_start(
                out=ci_sb[:].rearrange("p (s t) -> p s t", s=2),
                in_=ci_in[:, :, :].rearrange("s p t -> p s t"))
            for d in range(2):
                x_src = x_dis if d else x_drug
                idx_t = idxp.tile([128, R * NT * C], i32, tag="idx")
                nc.sync.dma_start(out=idx_t[:], in_=srcidx_in[d, :, :])
                dl_t = idxp.tile([128, R * NT * C], bf16, tag="dl")
                nc.sync.dma_start(out=dl_t[:], in_=dstloc_in[d, :, :])
                for t in range(NT):
                    rows = slice(t * 128, (t + 1) * 128)
                    ci_col = d * NT + t
                    o2 = ps2.tile([128, R * OUT], f32, tag="o2")
                    for r in range(R):
                        col0 = (r * NT + t) * C
                        # the HW indirect DMA supports exactly one gathered row
                        # per partition per instruction -> one gather per chunk
                        p_t = sb.tile([128, C * 128], bf16, tag="p")
                        nc.vector.tensor_tensor(
                            out=p_t[:].rearrange("p (c f) -> p c f", c=C),
                            in0=dl_t[:, col0:col0 + C][:, :, None].to_broadcast(
                                [128, C, 128]),
                            in1=iota_t[:, None, :].to_broadcast([128, C, 128]),
                            op=eq,
                        )
                        zt = ps.tile([128, 128], f32, tag="zt")
                        for j in range(C):
                            m_t = sb.tile([128, 128], bf16, tag="m")
                            nc.gpsimd.indirect_dma_start(
                                out=m_t[:], out_offset=None,
                                in_=x_src[:, :],
                                in_offset=bass.IndirectOffsetOnAxis(
                                    ap=idx_t[:, col0 + j:col0 + j + 1], axis=0),
                            )
                            nc.tensor.matmul(
                                out=zt[:], lhsT=m_t[:],
                                rhs=p_t[:, j * 128:(j + 1) * 128],
                                start=(j == 0), stop=(j == C - 1),
                            )
                        zt_sb = sb.tile([128, 128], bf16, tag="ztsb")
                        nc.scalar.copy(out=zt_sb[:], in_=zt[:])
                        nc.tensor.matmul(
                            out=o2[:, r * OUT:(r + 1) * OUT], lhsT=zt_sb[:],
                            rhs=wfc[:, r * OUT:(r + 1) * OUT],
                            start=True, stop=True,
                        )
                        if DEBUG_TAPS and d == 0 and t == 0 and r == 0:
                            nc.sync.dma_start(out=dbg["p"][:, :], in_=p_t[:])
                            nc.sync.dma_start(out=dbg["zt"][:, :], in_=zt_sb[:])
                    if DEBUG_TAPS and d == 0 and t == 0:
                        nc.vector.tensor_copy(out=dbg_o2_sb[:], in_=o2[:])
                        nc.sync.dma_start(out=dbg["o2"][:, :], in_=dbg_o2_sb[:])
                    ob = sb.tile([128, R * OUT], f32, tag="ob")
                    nc.vector.tensor_tensor(
                        out=ob[:], in0=o2[:],
                        in1=ci_sb[:, ci_col:ci_col + 1].to_broadcast(
                            [128, R * OUT]),
                        op=mult,
                    )
                    nc.vector.tensor_tensor(
                        out=ob[:], in0=ob[:], in1=bias5[:], op=add)
                    nc.sync.dma_start(
                        out=out[d, rows, :, :].rearrange("p r o -> p (r o)"),
                        in_=ob[:],
                    )
    return nc


# ======================================================================
# kernel entry
# ======================================================================

_cache: dict = {}


def kernel(drug_feat, dis_feat, cj_drug, ci_drug, cj_dis, ci_dis,
           att, basis, fc_w, fc_b, edge_drug, edge_dis):
    NPC, NT, NPAD, TBL = _derived()
    drug_feat = np.asarray(drug_feat, np.float32)
    dis_feat = np.asarray(dis_feat, np.float32)
    cj_drug = np.asarray(cj_drug, np.float32)
    ci_drug = np.asarray(ci_drug, np.float32)
    cj_dis = np.asarray(cj_dis, np.float32)
    ci_dis = np.asarray(ci_dis, np.float32)
    att = np.asarray(att, np.float32)
    basis = np.asarray(basis, np.float32)
    fc_w = np.asarray(fc_w, np.float32)
    fc_b = np.asarray(fc_b, np.float32)
    edge_drug = np.asarray(edge_drug, np.int32)
    edge_dis = np.asarray(edge_dis, np.int32)

    # ---- host preprocessing: edge sort/shard (index manipulation only) ----
    # direction 0: drug -> dis (dest = dis), direction 1: dis -> drug
    _tlog("start")
    _, _, c0 = _prep_direction(edge_drug, edge_dis, None)
    _, _, c1 = _prep_direction(edge_dis, edge_drug, None)
    C = max(c0, c1)
    src0, dl0, _ = _prep_direction(edge_drug, edge_dis, C)
    src1, dl1, _ = _prep_direction(edge_dis, edge_drug, C)
    _tlog("host prep done")

    # ---- launch 1: build gather tables ----
    if "prep" not in _cache:
        _cache["prep"] = build_prep_nc()
    nc1 = _cache["prep"]

    in_maps1 = []
    for c in range(NCORES):
        rows = slice(c * NPC, (c + 1) * NPC)
        feat_slice = np.zeros((2, NPAD, F), np.float32)
        feat_slice[0, :NPC] = drug_feat[rows]
        feat_slice[1, :NPC] = dis_feat[rows]
        cj_slice = np.zeros((2, NPAD), np.float32)
        cj_slice[0, :NPC] = cj_drug[rows]
        cj_slice[1, :NPC] = cj_dis[rows]
        cj_slice = np.ascontiguousarray(
            cj_slice.reshape(2, NT, 128).transpose(0, 2, 1))
        in_maps1.append({"feat_slice": feat_slice, "cj_slice": cj_slice})
    _tlog("launch1 inputs built")
    res1 = run_bass_kernel_spmd(nc1, in_maps1, core_ids=list(range(NCORES)))
    _tlog("launch1 done")
    xs = [r["x_slice"] for r in res1.results]
    x_drug_full = np.zeros((TBL, F), BF16)
    x_dis_full = np.zeros((TBL, F), BF16)
    for c in range(NCORES):
        rows = slice(c * NPC, (c + 1) * NPC)
        x_drug_full[rows] = xs[c][0, :NPC]
        x_dis_full[rows] = xs[c][1, :NPC]

    # ---- launch 2: main ----
    key = ("main", C)
    if key not in _cache:
        _cache[key] = build_main_nc(C)
    nc2 = _cache[key]

    in_maps2 = []
    for c in range(NCORES):
        rows = slice(c * NPC, (c + 1) * NPC)
        srcidx = np.stack([src0[c], src1[c]], axis=0)
        dstloc = np.stack([dl0[c], dl1[c]], axis=0)
        ci_pad = np.zeros((2, NPAD), np.float32)
        ci_pad[0, :NPC] = ci_dis[rows]    # dir 0 dest = dis
        ci_pad[1, :NPC] = ci_drug[rows]   # dir 1 dest = drug
        ci_pad = np.ascontiguousarray(
            ci_pad.reshape(2, NT, 128).transpose(0, 2, 1))
        in_maps2.append({
            "x_drug": x_drug_full, "x_dis": x_dis_full,
            "srcidx": srcidx, "dstloc": dstloc, "ci_pad": ci_pad,
            "att": att, "basis": basis, "fc_w": fc_w, "fc_b": fc_b,
        })
    _tlog("launch2 inputs built")
    res2 = run_bass_kernel_spmd(nc2, in_maps2, core_ids=list(range(NCORES)))
    _tlog("launch2 done")

    out_dis = np.concatenate(
        [r["out_part"][0, :NPC] for r in res2.results], axis=0)
    out_drug = np.concatenate(
        [r["out_part"][1, :NPC] for r in res2.results], axis=0)
    _tlog("assembled")
    return out_drug.astype(np.float32), out_dis.astype(np.float32)


# revision 30
# speedup vs baseline: 4173.3763x; 11.4253x over previous
"""GCMC layer (gnn_message_passing) Bass kernel for 8 Trainium2 NeuronCores.

Strategy (dest-sharded, no collectives):
  out_dis[m, r, :] = ci_dis[m] * (S_dis[r][m] @ Wfc_r) + fc_b
  where S_dis[r][m] = sum_{edges e of rating r with dst=m} x_drug[src[e]]
        x_drug[n]   = cj_drug[n] * drug_feat[n]      (bf16 gather table)
        Wfc_r       = (sum_b att[r,b]*basis[b]) @ fc_w    [F, OUT]
  (and symmetrically for the reverse direction dis->drug)

  - Host sorts edges of each (direction, rating) by destination, shards
    destinations across 8 cores, and lays out per-dest-tile (128 dests)
    edge chunks of 128, padded to a static chunk count C with edges that
    point at an all-zero table row.
  - Launch 1: each core scales its 1/8 slice of node features by cj -> bf16.
    Host concatenates the slices into full gather tables.
  - Launch 2 (main): per dest tile: indirect-DMA gather of message rows
    [128, C, F], one-hot P = is_equal(dstloc, iota) on DVE, TensorE
    accumulates ZT[f, d] += M_chunk.T @ P_chunk in PSUM (the segment sum),
    second matmul ZT.T @ Wfc_r into a per-tile [128, R*OUT] PSUM bank,
    then one scalar_tensor_tensor applies ci scale + bias, and the result
    is stored contiguously in the final [node, r, out] layout.
"""

import json
import os
import time

import numpy as np
import ml_dtypes

_VERBOSE = os.environ.get("KERNEL_VERBOSE", "0") == "1"


def _tlog(msg, t0=[None]):
    if _VERBOSE:
        now = time.time()
        dt = 0.0 if t0[0] is None else now - t0[0]
        t0[0] = now
        print(f"[kernel +{dt:6.2f}s] {msg}", flush=True)

import concourse.bass as bass
import concourse.mybir as mybir
import concourse.tile as tile
from concourse.bass_utils import run_bass_kernel_spmd

BF16 = ml_dtypes.bfloat16


# ----------------------------------------------------------------------
# Workaround: the staged walrus rejects >1 sync wait per instruction
# ("Too many sync wait commands") while the Tile scheduler emits multi-wait
# instructions.  Split extra waits into standalone EventSemaphore
# instructions right before the owning instruction (same engine queue, so
# semantics are identical: all waits are pre-conditions).
# ----------------------------------------------------------------------

def _split_multiwaits(bir: bytes) -> bytes:
    j = json.loads(bir)
    for fn in j["functions"]:
        for blk in fn["blocks"]:
            out = []
            k = 0
            for ins in blk["instructions"]:
                si = ins.get("sync_info") or {}
                waits = si.get("on_wait") or []
                if len(waits) > 1:
                    for w in waits[:-1]:
                        out.append({
                            "debug": ins.get("debug"),
                            "engine": ins["engine"],
                            "ins": [], "outs": [],
                            "name": f"{ins['name']}-ws{k}",
                            "opcode": "EventSemaphore",
                            "sync_info": {"on_update": [], "on_wait": [w]},
                        })
                        k += 1
                    si["on_wait"] = [waits[-1]]
                out.append(ins)
            blk["instructions"] = out
    return json.dumps(j).encode()


_orig_to_json_bytes = bass.Bass.to_json_bytes


def _patched_to_json_bytes(self):
    return _split_multiwaits(_orig_to_json_bytes(self))


bass.Bass.to_json_bytes = _patched_to_json_bytes

# ----- problem constants (hardcoded per contract) -----
N = 50000          # nodes per side
F = 128            # feature dim
R = 5              # ratings
E = 400000         # edges per rating per direction
OUT = 64           # output dim
NB = 2             # basis count
NCORES = 8

f32 = mybir.dt.float32
bf16 = mybir.dt.bfloat16
i32 = mybir.dt.int32


def _derived():
    npc = N // NCORES
    nt = (npc + 127) // 128
    npad = nt * 128
    tbl = ((N + 1 + 127) // 128) * 128  # >= N+1 so row N exists and is zero
    return npc, nt, npad, tbl


# ======================================================================
# Host-side edge preprocessing
# ======================================================================

def _prep_direction(src_all: np.ndarray, dst_all: np.ndarray, C: int | None):
    """For one direction: returns (srcidx, dstloc, maxc) where
    srcidx: int32 [NCORES, 128, R*NT*C]   gather indices into the src table
    dstloc: bf16  [NCORES, 128, R*NT*C]   dest offset (0..127) within tile
    Column (r*NT + t)*C + j, partition i  <->  edge slot j*128+i of tile t.
    If C is None, only computes the needed max chunk count.
    """
    NPC, NT, _, _ = _derived()
    ZROW = N
    per_core = []
    maxc = 1
    for r in range(R):
        order = np.argsort(dst_all[r], kind="stable")
        dst_s = dst_all[r][order].astype(np.int64)
        src_s = src_all[r][order].astype(np.int64)
        bounds = np.searchsorted(dst_s, np.arange(NCORES + 1) * NPC)
        for c in range(NCORES):
            lo, hi = bounds[c], bounds[c + 1]
            d = dst_s[lo:hi] - c * NPC
            s = src_s[lo:hi]
            tile_id = d >> 7
            tcnt = np.bincount(tile_id, minlength=NT)
            maxc = max(maxc, int((tcnt.max() + 127) // 128))
            if C is not None:
                per_core.append((r, c, d, s, tile_id, tcnt))
    if C is None:
        return None, None, maxc
    assert maxc <= C, f"need C>={maxc}, got {C}"

    srcidx = np.full((NCORES, R, NT, C * 128), ZROW, np.int32)
    dstloc = np.zeros((NCORES, R, NT, C * 128), np.float32)
    for (r, c, d, s, tile_id, tcnt) in per_core:
        tstart = np.zeros(NT + 1, np.int64)
        np.cumsum(tcnt, out=tstart[1:])
        rank = np.arange(len(d)) - tstart[tile_id]
        srcidx[c, r, tile_id, rank] = s
        dstloc[c, r, tile_id, rank] = (d & 127).astype(np.float32)
    # [c, r, NT, C, 128] -> [c, 128, r, NT, C] -> [c, 128, R*NT*C]
    srcidx = np.ascontiguousarray(
        srcidx.reshape(NCORES, R, NT, C, 128).transpose(0, 4, 1, 2, 3)
    ).reshape(NCORES, 128, R * NT * C)
    dstloc = np.ascontiguousarray(
        dstloc.reshape(NCORES, R, NT, C, 128).transpose(0, 4, 1, 2, 3)
    ).reshape(NCORES, 128, R * NT * C).astype(BF16)
    return srcidx, dstloc, maxc


# ======================================================================
# Launch 1: build bf16 gather tables (x = cj * feat), row-sharded
# ======================================================================

def build_prep_nc():
    NPC, NT, NPAD, TBL = _derived()
    nc = bass.Bass()
    feat_in = nc.dram_tensor("feat_slice", (2, NPAD, F), f32, kind="ExternalInput")
    # cj host-transposed to [2, 128, NT]: element [s, i, t] = cj[s, t*128+i]
    cj_in = nc.dram_tensor("cj_slice", (2, 128, NT), f32, kind="ExternalInput")
    x_out = nc.dram_tensor("x_slice", (2, NPAD, F), bf16, kind="ExternalOutput")

    with tile.TileContext(nc) as tc:
        with (
            tc.tile_pool(name="cj", bufs=1) as cjp,
            tc.tile_pool(name="sb", bufs=6) as sb,
        ):
            cj_sb = cjp.tile([128, 2 * NT], f32, tag="cj")
            nc.sync.dma_start(
                out=cj_sb[:].rearrange("p (s t) -> p s t", s=2),
                in_=cj_in[:, :, :].rearrange("s p t -> p s t"))
            # absorber: advance DVE's clock past the cj DMA so later consumers
            # need only one wait (walrus allows a single sync wait per compute
            # instruction and the scheduler doesn't always split).
            scratch = cjp.tile([128, 1], f32, tag="scratch")
            nc.vector.tensor_copy(out=scratch[:], in_=cj_sb[:, :1])
            for side in range(2):
                for t in range(NT):
                    rows = slice(t * 128, (t + 1) * 128)
                    ft = sb.tile([128, F], f32, tag="ft")
                    nc.sync.dma_start(out=ft[:], in_=feat_in[side, rows, :])
                    xt = sb.tile([128, F], bf16, tag="xt")
                    c0 = side * NT + t
                    # tensor_tensor (not tensor_scalar): the TS ISA struct only
                    # fits one sync wait and the scheduler may attach two.
                    nc.vector.tensor_tensor(
                        out=xt[:], in0=ft[:],
                        in1=cj_sb[:, c0:c0 + 1].to_broadcast([128, F]),
                        op=mybir.AluOpType.mult,
                    )
                    nc.sync.dma_start(out=x_out[side, rows, :], in_=xt[:])
    return nc


# ======================================================================
# Launch 2: main kernel
# ======================================================================

DEBUG_TAPS = False


def build_main_nc(C: int):
    NPC, NT, NPAD, TBL = _derived()
    nc = bass.Bass()
    dbg = {}
    if DEBUG_TAPS:
        dbg["m"] = nc.dram_tensor("dbg_m", (128, C * 128), bf16, kind="ExternalOutput")
        dbg["p"] = nc.dram_tensor("dbg_p", (128, C * 128), bf16, kind="ExternalOutput")
        dbg["zt"] = nc.dram_tensor("dbg_zt", (128, 128), bf16, kind="ExternalOutput")
        dbg["wfc"] = nc.dram_tensor("dbg_wfc", (128, R * OUT), bf16, kind="ExternalOutput")
        dbg["o2"] = nc.dram_tensor("dbg_o2", (128, R * OUT), f32, kind="ExternalOutput")
    x_drug = nc.dram_tensor("x_drug", (TBL, F), bf16, kind="ExternalInput")
    x_dis = nc.dram_tensor("x_dis", (TBL, F), bf16, kind="ExternalInput")
    srcidx_in = nc.dram_tensor("srcidx", (2, 128, R * NT * C), i32, kind="ExternalInput")
    dstloc_in = nc.dram_tensor("dstloc", (2, 128, R * NT * C), bf16, kind="ExternalInput")
    # ci host-transposed to [2, 128, NT]
    ci_in = nc.dram_tensor("ci_pad", (2, 128, NT), f32, kind="ExternalInput")
    att_in = nc.dram_tensor("att", (R, NB), f32, kind="ExternalInput")
    basis_in = nc.dram_tensor("basis", (NB, F, F), f32, kind="ExternalInput")
    fcw_in = nc.dram_tensor("fc_w", (F, OUT), f32, kind="ExternalInput")
    fcb_in = nc.dram_tensor("fc_b", (OUT,), f32, kind="ExternalInput")
    out = nc.dram_tensor("out_part", (2, NPAD, R, OUT), f32, kind="ExternalOutput")

    iota_np = np.broadcast_to(np.arange(128, dtype=np.float32), (128, 128))
    iota_c = nc.inline_tensor(np.ascontiguousarray(iota_np), "iota_c")
    ident_c = nc.inline_tensor(np.eye(128, dtype=np.float32), "ident_c")
    ones_c = nc.inline_tensor(np.ones((1, 128), dtype=np.float32), "ones_c")

    eq = mybir.AluOpType.is_equal
    mult = mybir.AluOpType.mult
    add = mybir.AluOpType.add

    with tile.TileContext(nc) as tc:
        with (
            tc.tile_pool(name="const", bufs=1) as cp,
            tc.tile_pool(name="sb", bufs=4) as sb,
            tc.tile_pool(name="mp", bufs=24) as mp,
            tc.tile_pool(name="pp", bufs=6) as pp,
            tc.tile_pool(name="idxp", bufs=2) as idxp,
            tc.tile_pool(name="ps", bufs=6, space="PSUM") as ps,
            tc.tile_pool(name="ps2", bufs=2, space="PSUM") as ps2,
        ):
            # ---------- consts ----------
            iota_f = cp.tile([128, 128], f32, tag="iotaf")
            nc.sync.dma_start(out=iota_f[:], in_=iota_c[:, :])
            iota_t = cp.tile([128, 128], bf16, tag="iota")
            nc.vector.tensor_copy(out=iota_t[:], in_=iota_f[:])
            ident_t = cp.tile([128, 128], f32, tag="ident")
            nc.sync.dma_start(out=ident_t[:], in_=ident_c[:, :])
            ones_f32 = cp.tile([1, 128], f32, tag="ones32")
            nc.sync.dma_start(out=ones_f32[:], in_=ones_c[:, :])
            fcw_t = cp.tile([128, OUT], f32, tag="fcw")
            nc.sync.dma_start(out=fcw_t[:], in_=fcw_in[:, :])
            fcb_row = cp.tile([1, OUT], f32, tag="fcbrow")
            nc.sync.dma_start(out=fcb_row[:], in_=fcb_in[None, :])
            att_row = cp.tile([1, R * NB], f32, tag="attrow")
            nc.sync.dma_start(out=att_row[:],
                              in_=att_in[:, :].rearrange("r b -> () (r b)"))

            # ---------- W prep ----------
            # broadcast att and bias across partitions via K=1 matmuls
            attb_ps = ps2.tile([128, R * NB], f32, tag="o2")
            nc.tensor.matmul(out=attb_ps[:], lhsT=ones_f32[:], rhs=att_row[:],
                             start=True, stop=True)
            att_b = cp.tile([128, R * NB], f32, tag="attb")
            nc.vector.tensor_copy(out=att_b[:], in_=attb_ps[:])

            biasb_ps = ps2.tile([128, OUT], f32, tag="o2")
            nc.tensor.matmul(out=biasb_ps[:], lhsT=ones_f32[:], rhs=fcb_row[:],
                             start=True, stop=True)
            bias5 = cp.tile([128, R * OUT], f32, tag="bias5")
            for r in range(R):
                nc.vector.tensor_copy(out=bias5[:, r * OUT:(r + 1) * OUT],
                                      in_=biasb_ps[:])

            # basis[b] transposed: [e, f]
            bT = []
            for b in range(NB):
                bt_in = sb.tile([128, 128], f32, tag="bload")
                nc.sync.dma_start(out=bt_in[:], in_=basis_in[b, :, :])
                bt_ps = ps.tile([128, 128], f32, tag="zt")
                nc.tensor.transpose(out=bt_ps[:], in_=bt_in[:], identity=ident_t[:])
                bt_sb = cp.tile([128, 128], f32, tag=f"bT{b}")
                nc.vector.tensor_copy(out=bt_sb[:], in_=bt_ps[:])
                bT.append(bt_sb)

            wfc = cp.tile([128, R * OUT], bf16, tag="wfc")
            for r in range(R):
                wrt = sb.tile([128, 128], f32, tag="wrt")
                tmp = sb.tile([128, 128], f32, tag="wtmp")
                nc.vector.tensor_tensor(
                    out=tmp[:], in0=bT[1][:],
                    in1=att_b[:, 2 * r + 1:2 * r + 2].to_broadcast([128, 128]),
                    op=mult,
                )
                nc.vector.tensor_tensor(
                    out=wrt[:], in0=bT[0][:],
                    in1=att_b[:, 2 * r:2 * r + 1].to_broadcast([128, 128]),
                    op=mult,
                )
                nc.vector.tensor_tensor(out=wrt[:], in0=wrt[:], in1=tmp[:], op=add)
                wfc_ps = ps2.tile([128, OUT], f32, tag="o2")
                nc.tensor.matmul(out=wfc_ps[:], lhsT=wrt[:], rhs=fcw_t[:],
                                 start=True, stop=True)
                nc.scalar.copy(out=wfc[:, r * OUT:(r + 1) * OUT], in_=wfc_ps[:])

            # ---------- main loops ----------
            if DEBUG_TAPS:
                nc.sync.dma_start(out=dbg["wfc"][:, :], in_=wfc[:])
                dbg_o2_sb = cp.tile([128, R * OUT], f32, tag="dbgo2")
            ci_sb = cp.tile([128, 2 * NT], f32, tag="ci")
            nc.sync.dma_start(
                out=ci_sb[:].rearrange("p (s t) -> p s t", s=2),
                in_=ci_in[:, :, :].rearrange("s p t -> p s t"))
            for d in range(2):
                x_src = x_dis if d else x_drug
                idx_t = idxp.tile([128, R * NT * C], i32, tag="idx")
                nc.sync.dma_start(out=idx_t[:], in_=srcidx_in[d, :, :])
                dl_t = idxp.tile([128, R * NT * C], bf16, tag="dl")
                nc.sync.dma_start(out=dl_t[:], in_=dstloc_in[d, :, :])
                for t in range(NT):
                    rows = slice(t * 128, (t + 1) * 128)
                    ci_col = d * NT + t
                    o2 = ps2.tile([128, R * OUT], f32, tag="o2")
                    for r in range(R):
                        col0 = (r * NT + t) * C
                        # the HW indirect DMA supports exactly one gathered row
                        # per partition per instruction -> one gather per chunk
                        p_t = pp.tile([128, C * 128], bf16, tag="p")
                        nc.vector.tensor_tensor(
                            out=p_t[:].rearrange("p (c f) -> p c f", c=C),
                            in0=dl_t[:, col0:col0 + C][:, :, None].to_broadcast(
                                [128, C, 128]),
                            in1=iota_t[:, None, :].to_broadcast([128, C, 128]),
                            op=eq,
                        )
                        zt = ps.tile([128, 128], f32, tag="zt")
                        for j in range(C):
                            m_t = mp.tile([128, 128], bf16, tag="m")
                            nc.gpsimd.indirect_dma_start(
                                out=m_t[:], out_offset=None,
                                in_=x_src[:, :],
                                in_offset=bass.IndirectOffsetOnAxis(
                                    ap=idx_t[:, col0 + j:col0 + j + 1], axis=0),
                            )
                            nc.tensor.matmul(
                                out=zt[:], lhsT=m_t[:],
                                rhs=p_t[:, j * 128:(j + 1) * 128],
                                start=(j == 0), stop=(j == C - 1),
                            )
                        zt_sb = pp.tile([128, 128], bf16, tag="ztsb")
                        nc.scalar.copy(out=zt_sb[:], in_=zt[:])
                        nc.tensor.matmul(
                            out=o2[:, r * OUT:(r + 1) * OUT], lhsT=zt_sb[:],
                            rhs=wfc[:, r * OUT:(r + 1) * OUT],
                            start=True, stop=True,
                        )
                        if DEBUG_TAPS and d == 0 and t == 0 and r == 0:
                            nc.sync.dma_start(out=dbg["p"][:, :], in_=p_t[:])
                            nc.sync.dma_start(out=dbg["zt"][:, :], in_=zt_sb[:])
                    if DEBUG_TAPS and d == 0 and t == 0:
                        nc.vector.tensor_copy(out=dbg_o2_sb[:], in_=o2[:])
                        nc.sync.dma_start(out=dbg["o2"][:, :], in_=dbg_o2_sb[:])
                    ob = sb.tile([128, R * OUT], f32, tag="ob")
                    nc.vector.tensor_tensor(
                        out=ob[:], in0=o2[:],
                        in1=ci_sb[:, ci_col:ci_col + 1].to_broadcast(
                            [128, R * OUT]),
                        op=mult,
                    )
                    nc.vector.tensor_tensor(
                        out=ob[:], in0=ob[:], in1=bias5[:], op=add)
                    nc.sync.dma_start(
                        out=out[d, rows, :, :].rearrange("p r o -> p (r o)"),
                        in_=ob[:],
                    )
    return nc


# ======================================================================
# kernel entry
# ======================================================================

_cache: dict = {}


def kernel(drug_feat, dis_feat, cj_drug, ci_drug, cj_dis, ci_dis,
           att, basis, fc_w, fc_b, edge_drug, edge_dis):
    NPC, NT, NPAD, TBL = _derived()
    drug_feat = np.asarray(drug_feat, np.float32)
    dis_feat = np.asarray(dis_feat, np.float32)
    cj_drug = np.asarray(cj_drug, np.float32)
    ci_drug = np.asarray(ci_drug, np.float32)
    cj_dis = np.asarray(cj_dis, np.float32)
    ci_dis = np.asarray(ci_dis, np.float32)
    att = np.asarray(att, np.float32)
    basis = np.asarray(basis, np.float32)
    fc_w = np.asarray(fc_w, np.float32)
    fc_b = np.asarray(fc_b, np.float32)
    edge_drug = np.asarray(edge_drug, np.int32)
    edge_dis = np.asarray(edge_dis, np.int32)

    # ---- host preprocessing: edge sort/shard (index manipulation only) ----
    # direction 0: drug -> dis (dest = dis), direction 1: dis -> drug
    _tlog("start")
    _, _, c0 = _prep_direction(edge_drug, edge_dis, None)
    _, _, c1 = _prep_direction(edge_dis, edge_drug, None)
    C = max(c0, c1)
    src0, dl0, _ = _prep_direction(edge_drug, edge_dis, C)
    src1, dl1, _ = _prep_direction(edge_dis, edge_drug, C)
    _tlog("host prep done")

    # ---- launch 1: build gather tables ----
    if "prep" not in _cache:
        _cache["prep"] = build_prep_nc()
    nc1 = _cache["prep"]

    in_maps1 = []
    for c in range(NCORES):
        rows = slice(c * NPC, (c + 1) * NPC)
        feat_slice = np.zeros((2, NPAD, F), np.float32)
        feat_slice[0, :NPC] = drug_feat[rows]
        feat_slice[1, :NPC] = dis_feat[rows]
        cj_slice = np.zeros((2, NPAD), np.float32)
        cj_slice[0, :NPC] = cj_drug[rows]
        cj_slice[1, :NPC] = cj_dis[rows]
        cj_slice = np.ascontiguousarray(
            cj_slice.reshape(2, NT, 128).transpose(0, 2, 1))
        in_maps1.append({"feat_slice": feat_slice, "cj_slice": cj_slice})
    _tlog("launch1 inputs built")
    res1 = run_bass_kernel_spmd(nc1, in_maps1, core_ids=list(range(NCORES)))
    _tlog("launch1 done")
    xs = [r["x_slice"] for r in res1.results]
    x_drug_full = np.zeros((TBL, F), BF16)
    x_dis_full = np.zeros((TBL, F), BF16)
    for c in range(NCORES):
        rows = slice(c * NPC, (c + 1) * NPC)
        x_drug_full[rows] = xs[c][0, :NPC]
        x_dis_full[rows] = xs[c][1, :NPC]

    # ---- launch 2: main ----
    key = ("main", C)
    if key not in _cache:
        _cache[key] = build_main_nc(C)
    nc2 = _cache[key]

    in_maps2 = []
    for c in range(NCORES):
        rows = slice(c * NPC, (c + 1) * NPC)
        srcidx = np.stack([src0[c], src1[c]], axis=0)
        dstloc = np.stack([dl0[c], dl1[c]], axis=0)
        ci_pad = np.zeros((2, NPAD), np.float32)
        ci_pad[0, :NPC] = ci_dis[rows]    # dir 0 dest = dis
        ci_pad[1, :NPC] = ci_drug[rows]   # dir 1 dest = drug
        ci_pad = np.ascontiguousarray(
            ci_pad.reshape(2, NT, 128).transpose(0, 2, 1))
        in_maps2.append({
            "x_drug": x_drug_full, "x_dis": x_dis_full,
            "srcidx": srcidx, "dstloc": dstloc, "ci_pad": ci_pad,
            "att": att, "basis": basis, "fc_w": fc_w, "fc_b": fc_b,
        })
    _tlog("launch2 inputs built")
    res2 = run_bass_kernel_spmd(nc2, in_maps2, core_ids=list(range(NCORES)))
    _tlog("launch2 done")

    out_dis = np.concatenate(
        [r["out_part"][0, :NPC] for r in res2.results], axis=0)
    out_drug = np.concatenate(
        [r["out_part"][1, :NPC] for r in res2.results], axis=0)
    _tlog("assembled")
    return out_drug.astype(np.float32), out_dis.astype(np.float32)
